# revision 1
# baseline (speedup 1.0000x reference)
"""Causal GQA self-attention (B=4,T=2048,C=2048,H=16,HKV=4,D=128) on 8 trn2 cores.

Sharding: core c -> (batch b = c//2, kv-group g = c%2). Each core computes the
attention output for its batch restricted to its 8 query heads (2 kv heads),
then the partial output projection against the matching 1024 rows of Wc.
Host sums the two partial outputs per batch. No collectives.

All matmuls run in bf16 (fp32 PSUM accumulation). Softmax skips the max
subtraction: rmsnormed q,k give |scores/sqrt(D)| <= sqrt(D) ~= 11.3, safely
inside fp32/exp range. Row sums come from a ones-column matmul on the same
bf16 probabilities used for P@V, normalization happens on the [d, q] output
tile (128x smaller than the score matrix).
"""

import math
import sys

import numpy as np

sys.path.insert(0, "/opt/trn_rl_repo")

import ml_dtypes

import concourse.bass as bass
import concourse.mybir as mybir
import concourse.tile as tile
from concourse import bacc
from concourse.bass_utils import run_bass_kernel_spmd

BF16 = mybir.dt.bfloat16
FP32 = mybir.dt.float32
NPBF16 = ml_dtypes.bfloat16

B, T, C = 4, 2048, 2048
H, HKV, D = 16, 4, 128
LH, LKV = 8, 2          # local q heads / kv heads per core
NCT = C // 128          # contraction c-tiles
NTT = T // 128          # t-tiles of 128
NQT = T // 512          # q-tiles of 512
EPS = 1e-6
SOFTMAX_SCALE = 1.0 / math.sqrt(D)

ACT = mybir.ActivationFunctionType

LAST_RESULTS = None     # BassKernelResults of the most recent run (for test.py)
_CACHED_NC = None


def _build_module():
    nc = bacc.Bacc("TRN2", target_bir_lowering=False, debug=False, num_devices=8)

    xt_d = nc.dram_tensor("xt", [NTT, 128, C], BF16, kind="ExternalInput").ap()
    wq_d = nc.dram_tensor("wq", [128, NCT, LH * D], BF16, kind="ExternalInput").ap()
    wkv_d = nc.dram_tensor("wkv", [128, NCT, 2 * LKV * D], BF16, kind="ExternalInput").ap()
    wc_d = nc.dram_tensor("wc", [128, LH, C], BF16, kind="ExternalInput").ap()
    cs_d = nc.dram_tensor("cs", [NTT, 128, 2 * (LH + LKV) * 64], BF16, kind="ExternalInput").ap()
    mk_d = nc.dram_tensor("masks", [128, 4, 512], BF16, kind="ExternalInput").ap()
    id_d = nc.dram_tensor("ident", [128, 128], BF16, kind="ExternalInput").ap()
    out_d = nc.dram_tensor("outT", [C, T], FP32, kind="ExternalOutput").ap()

    NH = LH + LKV  # 10 heads normed/roped per t-tile

    with tile.TileContext(nc) as tc:
        consts = tc.alloc_tile_pool(name="consts", bufs=1)
        persist = tc.alloc_tile_pool(name="persist", bufs=1)

        ident_s = consts.tile([128, 128], BF16)
        nc.sync.dma_start(out=ident_s, in_=id_d)
        mask_s = consts.tile([128, 4, 512], BF16)
        nc.sync.dma_start(out=mask_s, in_=mk_d)
        ones_col = consts.tile([128, 1], BF16)
        nc.vector.memset(ones_col, 1.0)
        ones_row_f = consts.tile([1, 128], FP32)
        nc.vector.memset(ones_row_f, 1.0)
        ones_row = consts.tile([1, 128], mybir.dt.float32r)
        nc.vector.tensor_copy(out=ones_row, in_=ones_row_f)
        eps_s = consts.tile([128, 1], FP32)
        nc.vector.memset(eps_s, EPS)

        # persistent activations
        qt_all = persist.tile([128, LH, T], BF16)    # Q^T per head [d, t]
        kt_all = persist.tile([128, LKV, T], BF16)   # K^T per kv head [d, t]
        v_all = persist.tile([128, NTT, LKV * D], BF16)  # V per t-tile [t, d]
        yt_all = persist.tile([128, LH, T], BF16)    # attn out y^T [d, t]

        # ---------------- phase 1: projections + rmsnorm + rope + transpose
        with (
            tc.tile_pool(name="wpool", bufs=1) as wpool,
            tc.tile_pool(name="xpool", bufs=2) as xpool,
            tc.tile_pool(name="cspool", bufs=2) as cspool,
            tc.tile_pool(name="stage", bufs=2) as stage,
            tc.tile_pool(name="stats", bufs=3) as stats,
            tc.tile_pool(name="ps1", bufs=2, space="PSUM") as ps1,
            tc.tile_pool(name="pstp", bufs=2, space="PSUM") as pstp,
        ):
            wq_s = wpool.tile([128, NCT, LH * D], BF16)
            nc.sync.dma_start(out=wq_s, in_=wq_d)
            wkv_s = wpool.tile([128, NCT, 2 * LKV * D], BF16)
            nc.sync.dma_start(out=wkv_s, in_=wkv_d)

            for tt in range(NTT):
                xt_s = xpool.tile([128, NCT, 128], BF16)
                nc.sync.dma_start(
                    out=xt_s, in_=xt_d[tt].rearrange("p (a b) -> p a b", a=NCT)
                )
                qkv_ps = ps1.tile([128, 1536], FP32)
                for ct in range(NCT):
                    st = ct == 0
                    sp = ct == NCT - 1
                    nc.tensor.matmul(
                        qkv_ps[:, 0:512], lhsT=xt_s[:, ct, :],
                        rhs=wq_s[:, ct, 0:512], start=st, stop=sp,
                    )
                    nc.tensor.matmul(
                        qkv_ps[:, 512:1024], lhsT=xt_s[:, ct, :],
                        rhs=wq_s[:, ct, 512:1024], start=st, stop=sp,
                    )
                    nc.tensor.matmul(
                        qkv_ps[:, 1024:1536], lhsT=xt_s[:, ct, :],
                        rhs=wkv_s[:, ct, :], start=st, stop=sp,
                    )
                # V tile straight out (no norm): cols 1280:1536
                nc.vector.tensor_copy(out=v_all[:, tt, :], in_=qkv_ps[:, 1280:1536])

                # q/k to SBUF fp32 (one PSUM operand per instruction max)
                raw_st = stage.tile([128, NH, 128], FP32, tag="raw")
                nc.vector.tensor_copy(
                    out=raw_st, in_=qkv_ps[:, 0:1280].rearrange("p (h d) -> p h d", h=NH)
                )
                # rmsnorm stats for the 10 q/k heads
                sq_t = stage.tile([128, NH, 128], FP32, tag="sq")
                nc.vector.tensor_mul(sq_t, raw_st, raw_st)
                ssq = stats.tile([128, NH], FP32)
                nc.vector.tensor_reduce(
                    out=ssq, in_=sq_t, axis=mybir.AxisListType.X, op=mybir.AluOpType.add
                )
                rstd = stats.tile([128, NH], FP32)
                nc.scalar.activation(rstd, ssq, ACT.Sqrt, scale=1.0 / D, bias=eps_s)
                nc.vector.reciprocal(out=rstd, in_=rstd)

                qk_st = stage.tile([128, NH, 128], BF16, tag="qk")
                for i in range(NH):
                    nc.scalar.activation(
                        qk_st[:, i, :], raw_st[:, i, :],
                        ACT.Copy, scale=rstd[:, i:i + 1],
                    )

                # rope
                cs_s = cspool.tile([128, 2 * NH * 64], BF16)
                nc.sync.dma_start(out=cs_s, in_=cs_d[tt])
                cos_r = cs_s[:, 0:NH * 64].rearrange("p (h f) -> p h f", h=NH)
                sin_r = cs_s[:, NH * 64:].rearrange("p (h f) -> p h f", h=NH)
                qk_pair = qk_st.rearrange("p h (f two) -> p h f two", two=2)
                q_e, q_o = qk_pair[:, :, :, 0], qk_pair[:, :, :, 1]

                rp_st = stage.tile([128, NH, 128], BF16, tag="rope")
                rp_pair = rp_st.rearrange("p h (f two) -> p h f two", two=2)
                r_e, r_o = rp_pair[:, :, :, 0], rp_pair[:, :, :, 1]
                t1 = stage.tile([128, NH, 64], BF16, tag="t1")
                t2 = stage.tile([128, NH, 64], BF16, tag="t2")
                nc.vector.tensor_mul(t1, q_e, cos_r)
                nc.vector.tensor_mul(t2, q_o, sin_r)
                nc.vector.tensor_sub(r_e, t1, t2)
                t3 = stage.tile([128, NH, 64], BF16, tag="t3")
                t4 = stage.tile([128, NH, 64], BF16, tag="t4")
                nc.vector.tensor_mul(t3, q_e, sin_r)
                nc.vector.tensor_mul(t4, q_o, cos_r)
                nc.vector.tensor_add(r_o, t3, t4)

                # transpose to [d, t]
                for i in range(NH):
                    tp = pstp.tile([128, 128], BF16)
                    nc.tensor.transpose(tp, rp_st[:, i, :], ident_s)
                    dst = (
                        qt_all[:, i, tt * 128:(tt + 1) * 128]
                        if i < LH
                        else kt_all[:, i - LH, tt * 128:(tt + 1) * 128]
                    )
                    nc.vector.tensor_copy(out=dst, in_=tp)

        # ---------------- phase 2: attention + phase 3: out-projection
        with (
            tc.tile_pool(name="wcpool", bufs=1) as wcpool,
            tc.tile_pool(name="ppool", bufs=6) as ppool,
            tc.tile_pool(name="rcpool", bufs=2) as rcpool,
            tc.tile_pool(name="bcpool", bufs=2) as bcpool,
            tc.tile_pool(name="outst", bufs=3) as outst,
            tc.tile_pool(name="pss", bufs=2, space="PSUM") as pss,
            tc.tile_pool(name="pso", bufs=2, space="PSUM") as pso,
            tc.tile_pool(name="psr", bufs=1, space="PSUM") as psr,
            tc.tile_pool(name="psb", bufs=1, space="PSUM") as psb,
            tc.tile_pool(name="psop", bufs=2, space="PSUM") as psop,
        ):
            wc_s = wcpool.tile([128, LH, C], BF16)
            nc.sync.dma_start(out=wc_s, in_=wc_d)

            for lh in range(LH):
                lkv = lh // (H // HKV)
                for qt in range(NQT):
                    nkb = 4 * qt + 4
                    o_ps = pso.tile([128, 512], FP32)
                    rs_ps = psr.tile([1, 512], FP32)
                    for kb in range(nkb):
                        s_ps = pss.tile([128, 512], FP32)
                        nc.tensor.matmul(
                            s_ps,
                            lhsT=kt_all[:, lkv, kb * 128:(kb + 1) * 128],
                            rhs=qt_all[:, lh, qt * 512:(qt + 1) * 512],
                            start=True, stop=True,
                        )
                        pt = ppool.tile([128, 512], BF16)
                        nc.scalar.activation(pt, s_ps, ACT.Exp, scale=SOFTMAX_SCALE)
                        if kb >= 4 * qt:
                            nc.vector.tensor_mul(pt, pt, mask_s[:, kb - 4 * qt, :])
                        nc.tensor.matmul(
                            o_ps,
                            lhsT=v_all[:, kb, lkv * D:(lkv + 1) * D],
                            rhs=pt, start=(kb == 0), stop=(kb == nkb - 1),
                        )
                        nc.tensor.matmul(
                            rs_ps, lhsT=ones_col, rhs=pt,
                            start=(kb == 0), stop=(kb == nkb - 1),
                        )
                    rcp = rcpool.tile([1, 512], mybir.dt.float32r)
                    with nc.allow_low_precision(
                        reason="fp32r broadcast of reciprocal; full fp32 data path"
                    ):
                        nc.vector.reciprocal(out=rcp, in_=rs_ps)
                    bc_ps = psb.tile([128, 512], FP32)
                    nc.tensor.matmul(bc_ps, lhsT=ones_row, rhs=rcp, start=True, stop=True)
                    bc_sb = bcpool.tile([128, 512], FP32)
                    nc.scalar.copy(bc_sb, bc_ps)
                    nc.vector.tensor_mul(
                        yt_all[:, lh, qt * 512:(qt + 1) * 512], o_ps, bc_sb
                    )

            # out projection: outT[co, t] = sum_lc wc[:, lc, co].T @ yt[lc]
            for ct in range(C // 128):
                for t4 in range(NQT):
                    op_ps = psop.tile([128, 512], FP32)
                    for lc in range(LH):
                        nc.tensor.matmul(
                            op_ps,
                            lhsT=wc_s[:, lc, ct * 128:(ct + 1) * 128],
                            rhs=yt_all[:, lc, t4 * 512:(t4 + 1) * 512],
                            start=(lc == 0), stop=(lc == LH - 1),
                        )
                    ost = outst.tile([128, 512], FP32)
                    nc.scalar.copy(ost, op_ps)
                    nc.sync.dma_start(
                        out=out_d[ct * 128:(ct + 1) * 128, t4 * 512:(t4 + 1) * 512],
                        in_=ost,
                    )

        persist.release()
        consts.release()

    nc.compile()
    return nc


def _prep_inputs(x, freqs_cis, Wq, Wk, Wv, Wc):
    """Host-side shard + layout prep. Returns the 8 per-core input maps."""
    x = np.asarray(x, dtype=np.float32)
    freqs_cis = np.asarray(freqs_cis, dtype=np.float32)
    NH = LH + LKV

    # cos/sin replicated per local head: [NTT, 128, 2*NH*64]
    cos = np.tile(freqs_cis[:, :, 0], (1, NH))  # [T, NH*64]
    sin = np.tile(freqs_cis[:, :, 1], (1, NH))
    cs = np.concatenate([cos, sin], axis=1).reshape(NTT, 128, 2 * NH * 64)
    cs = np.ascontiguousarray(cs).astype(NPBF16)

    # causal masks for the 4 diagonal alignments: keep where (y - 128j - x) >= 0
    xg = np.arange(128)[:, None]
    yg = np.arange(512)[None, :]
    masks = np.stack(
        [(yg - 128 * j - xg >= 0) for j in range(4)], axis=1
    ).astype(NPBF16)  # [128, 4, 512]

    ident = np.eye(128, dtype=NPBF16)

    def tile_rows(w):  # [C, n] -> [128, NCT, n] with row = ct*128 + p
        return np.ascontiguousarray(
            w.reshape(NCT, 128, w.shape[1]).transpose(1, 0, 2)
        ).astype(NPBF16)

    in_maps = []
    xts = []
    for b in range(B):
        xt = x[b].T.reshape(NCT, 128, NTT, 128).transpose(2, 1, 0, 3)
        xts.append(np.ascontiguousarray(xt.reshape(NTT, 128, C)).astype(NPBF16))

    for core in range(8):
        b, g = core // 2, core % 2
        wq = tile_rows(Wq[:, g * LH * D:(g + 1) * LH * D])
        wk = Wk[:, g * LKV * D:(g + 1) * LKV * D]
        wv = Wv[:, g * LKV * D:(g + 1) * LKV * D]
        wkv = tile_rows(np.concatenate([wk, wv], axis=1))
        wc = np.ascontiguousarray(
            Wc[g * LH * D:(g + 1) * LH * D].reshape(LH, 128, C).transpose(1, 0, 2)
        ).astype(NPBF16)
        in_maps.append(
            {
                "xt": xts[b],
                "wq": wq,
                "wkv": wkv,
                "wc": wc,
                "cs": cs,
                "masks": masks,
                "ident": ident,
            }
        )
    return in_maps


def kernel(x, freqs_cis, Wq, Wk, Wv, Wc):
    global LAST_RESULTS, _CACHED_NC
    if _CACHED_NC is None:
        _CACHED_NC = _build_module()
    nc = _CACHED_NC
    in_maps = _prep_inputs(x, freqs_cis, Wq, Wk, Wv, Wc)
    res = run_bass_kernel_spmd(nc, in_maps, core_ids=list(range(8)))
    LAST_RESULTS = res
    out = np.empty((B, T, C), dtype=np.float32)
    for b in range(B):
        acc = res.results[2 * b]["outT"] + res.results[2 * b + 1]["outT"]
        out[b] = acc.T
    return out



# revision 6
# speedup vs baseline: 1.0544x; 1.0544x over previous
"""Causal GQA self-attention (B=4,T=2048,C=2048,H=16,HKV=4,D=128) on 8 trn2 cores.

Sharding: core c -> (batch b = c//2, kv-group g = c%2). Each core computes the
attention output for its batch restricted to its 8 query heads (2 kv heads),
then the partial output projection against the matching 1024 rows of Wc.
Host sums the two partial outputs per batch. No collectives.

All matmuls run in bf16 (fp32 PSUM accumulation). Softmax skips the max
subtraction: rmsnormed q,k give |scores/sqrt(D)| <= sqrt(D) ~= 11.3, safely
inside bf16/exp range.

Phase 2 layout: query heads are processed in pairs sharing one [128,1024]
PSUM tile (two 512-wide score matmuls, one exp activation over both halves).
The softmax denominator comes from a bf16 running sum of the probability
tiles on the DVE plus one gpsimd partition_all_reduce per (qt, head-pair) —
no PE cycles. Normalization is a DVE divide straight out of the PV PSUM.
The output projection is interleaved into the attention loop (it shares the
scores PSUM pool), giving the PE work while the Act engine paces the exps.
"""

import math
import sys

import numpy as np

sys.path.insert(0, "/opt/trn_rl_repo")

import ml_dtypes

import concourse.bass as bass
import concourse.bass_isa as bass_isa
import concourse.mybir as mybir
import concourse.tile as tile
from concourse import bacc
from concourse.bass_utils import run_bass_kernel_spmd

BF16 = mybir.dt.bfloat16
FP32 = mybir.dt.float32
NPBF16 = ml_dtypes.bfloat16

B, T, C = 4, 2048, 2048
H, HKV, D = 16, 4, 128
LH, LKV = 8, 2          # local q heads / kv heads per core
NCT = C // 128          # contraction c-tiles
NTT = T // 128          # t-tiles of 128
NQT = T // 512          # q-tiles of 512
EPS = 1e-6
SOFTMAX_SCALE = 1.0 / math.sqrt(D)

ACT = mybir.ActivationFunctionType
ALU = mybir.AluOpType

LAST_RESULTS = None     # BassKernelResults of the most recent run (for test.py)
_CACHED_NC = None


def _build_module():
    nc = bacc.Bacc("TRN2", target_bir_lowering=False, debug=False, num_devices=8)

    xt_d = nc.dram_tensor("xt", [NTT, 128, C], BF16, kind="ExternalInput").ap()
    wq_d = nc.dram_tensor("wq", [128, NCT, LH * D], BF16, kind="ExternalInput").ap()
    wkv_d = nc.dram_tensor("wkv", [128, NCT, 2 * LKV * D], BF16, kind="ExternalInput").ap()
    wc_d = nc.dram_tensor("wc", [128, LH, C], BF16, kind="ExternalInput").ap()
    cs_d = nc.dram_tensor("cs", [NTT, 128, 2 * (LH + LKV) * 64], BF16, kind="ExternalInput").ap()
    mk_d = nc.dram_tensor("masks", [128, 4, 1024], BF16, kind="ExternalInput").ap()
    id_d = nc.dram_tensor("ident", [128, 128], BF16, kind="ExternalInput").ap()
    out_d = nc.dram_tensor("outT", [C, T], FP32, kind="ExternalOutput").ap()

    NH = LH + LKV  # 10 heads normed/roped per t-tile

    with tile.TileContext(nc) as tc:
        consts = tc.alloc_tile_pool(name="consts", bufs=1)
        persist = tc.alloc_tile_pool(name="persist", bufs=1)

        ident_s = consts.tile([128, 128], BF16)
        nc.sync.dma_start(out=ident_s, in_=id_d)
        mask_s = consts.tile([128, 4, 1024], BF16)
        nc.sync.dma_start(out=mask_s, in_=mk_d)
        eps_s = consts.tile([128, 1], FP32)
        nc.vector.memset(eps_s, EPS)

        # persistent activations
        qt_all = persist.tile([128, LH, T], BF16)    # Q^T per head [d, t]
        kt_all = persist.tile([128, LKV, T], BF16)   # K^T per kv head [d, t]
        v_all = persist.tile([128, NTT, LKV * D], BF16)  # V per t-tile [t, d]
        yt_all = persist.tile([128, LH, T], BF16)    # attn out y^T [d, t]

        # ---------------- phase 1: projections + rmsnorm + rope + transpose
        with (
            tc.tile_pool(name="wpool", bufs=1) as wpool,
            tc.tile_pool(name="xpool", bufs=2) as xpool,
            tc.tile_pool(name="cspool", bufs=2) as cspool,
            tc.tile_pool(name="stage", bufs=2) as stage,
            tc.tile_pool(name="stats", bufs=3) as stats,
            tc.tile_pool(name="ps1", bufs=2, space="PSUM") as ps1,
            tc.tile_pool(name="pstp", bufs=2, space="PSUM") as pstp,
        ):
            wq_s = wpool.tile([128, NCT, LH * D], BF16)
            nc.sync.dma_start(out=wq_s, in_=wq_d)
            wkv_s = wpool.tile([128, NCT, 2 * LKV * D], BF16)
            nc.sync.dma_start(out=wkv_s, in_=wkv_d)

            for tt in range(NTT):
                xt_s = xpool.tile([128, NCT, 128], BF16)
                nc.sync.dma_start(
                    out=xt_s, in_=xt_d[tt].rearrange("p (a b) -> p a b", a=NCT)
                )
                qkv_ps = ps1.tile([128, 1536], FP32)
                for ct in range(NCT):
                    st = ct == 0
                    sp = ct == NCT - 1
                    nc.tensor.matmul(
                        qkv_ps[:, 0:512], lhsT=xt_s[:, ct, :],
                        rhs=wq_s[:, ct, 0:512], start=st, stop=sp,
                    )
                    nc.tensor.matmul(
                        qkv_ps[:, 512:1024], lhsT=xt_s[:, ct, :],
                        rhs=wq_s[:, ct, 512:1024], start=st, stop=sp,
                    )
                    nc.tensor.matmul(
                        qkv_ps[:, 1024:1536], lhsT=xt_s[:, ct, :],
                        rhs=wkv_s[:, ct, :], start=st, stop=sp,
                    )
                # V tile straight out (no norm): cols 1280:1536
                nc.vector.tensor_copy(out=v_all[:, tt, :], in_=qkv_ps[:, 1280:1536])

                # q/k to SBUF fp32 (one PSUM operand per instruction max)
                raw_st = stage.tile([128, NH, 128], FP32, tag="raw")
                nc.vector.tensor_copy(
                    out=raw_st, in_=qkv_ps[:, 0:1280].rearrange("p (h d) -> p h d", h=NH)
                )
                # rmsnorm stats for the 10 q/k heads
                sq_t = stage.tile([128, NH, 128], FP32, tag="sq")
                nc.vector.tensor_mul(sq_t, raw_st, raw_st)
                ssq = stats.tile([128, NH], FP32)
                nc.vector.tensor_reduce(
                    out=ssq, in_=sq_t, axis=mybir.AxisListType.X, op=ALU.add
                )
                rstd = stats.tile([128, NH], FP32)
                nc.scalar.activation(rstd, ssq, ACT.Sqrt, scale=1.0 / D, bias=eps_s)
                nc.vector.reciprocal(out=rstd, in_=rstd)

                qk_st = stage.tile([128, NH, 128], BF16, tag="qk")
                for i in range(NH):
                    nc.scalar.activation(
                        qk_st[:, i, :], raw_st[:, i, :],
                        ACT.Copy, scale=rstd[:, i:i + 1],
                    )

                # rope (rotate-half layout: host permuted Wq/Wk cols so the
                # even/odd pairs sit in contiguous halves [0:64]/[64:128])
                cs_s = cspool.tile([128, 2 * NH * 64], BF16)
                nc.sync.dma_start(out=cs_s, in_=cs_d[tt])
                cos_r = cs_s[:, 0:NH * 64].rearrange("p (h f) -> p h f", h=NH)
                sin_r = cs_s[:, NH * 64:].rearrange("p (h f) -> p h f", h=NH)
                qk_half = qk_st.rearrange("p h (two f) -> p h two f", two=2)
                q_e, q_o = qk_half[:, :, 0, :], qk_half[:, :, 1, :]

                rp_st = stage.tile([128, NH, 128], BF16, tag="rope")
                rp_half = rp_st.rearrange("p h (two f) -> p h two f", two=2)
                r_e, r_o = rp_half[:, :, 0, :], rp_half[:, :, 1, :]
                t1 = stage.tile([128, NH, 64], BF16, tag="t1")
                t2 = stage.tile([128, NH, 64], BF16, tag="t2")
                nc.vector.tensor_mul(t1, q_e, cos_r)
                nc.vector.tensor_mul(t2, q_o, sin_r)
                nc.vector.tensor_sub(r_e, t1, t2)
                t3 = stage.tile([128, NH, 64], BF16, tag="t3")
                t4 = stage.tile([128, NH, 64], BF16, tag="t4")
                nc.vector.tensor_mul(t3, q_e, sin_r)
                nc.vector.tensor_mul(t4, q_o, cos_r)
                nc.vector.tensor_add(r_o, t3, t4)

                # transpose to [d, t]
                for i in range(NH):
                    tp = pstp.tile([128, 128], BF16)
                    nc.tensor.transpose(tp, rp_st[:, i, :], ident_s)
                    dst = (
                        qt_all[:, i, tt * 128:(tt + 1) * 128]
                        if i < LH
                        else kt_all[:, i - LH, tt * 128:(tt + 1) * 128]
                    )
                    nc.vector.tensor_copy(out=dst, in_=tp)

        # ---------------- phase 2+3: attention with interleaved out-projection
        with (
            tc.tile_pool(name="wcpool", bufs=1) as wcpool,
            tc.tile_pool(name="ppool", bufs=4) as ppool,
            tc.tile_pool(name="accpool", bufs=2) as accpool,
            tc.tile_pool(name="dpool", bufs=2) as dpool,
            tc.tile_pool(name="outst", bufs=3) as outst,
            tc.tile_pool(name="pss", bufs=2, space="PSUM") as pss,
            tc.tile_pool(name="pso", bufs=2, space="PSUM") as pso,
        ):
            wc_s = wcpool.tile([128, LH, C], BF16)
            nc.sync.dma_start(out=wc_s, in_=wc_d)

            # out-projection for the 512-wide t-span t4, chunk of 4 c-tiles.
            # outT[co, t] = sum_lh wc[:, lh, co].T @ yt[lh]; two c-tiles share
            # one [128,1024] PSUM tile (independent 512-wide accumulations).
            def outproj_chunk(t4, ct0):
                for cpair in range(2):
                    op_ps = pss.tile([128, 1024], FP32, tag="s")
                    for half in range(2):
                        ct = ct0 + cpair * 2 + half
                        for lh in range(LH):
                            nc.tensor.matmul(
                                op_ps[:, half * 512:(half + 1) * 512],
                                lhsT=wc_s[:, lh, ct * 128:(ct + 1) * 128],
                                rhs=yt_all[:, lh, t4 * 512:(t4 + 1) * 512],
                                start=(lh == 0), stop=(lh == LH - 1),
                            )
                    ost = outst.tile([128, 1024], FP32)
                    nc.vector.tensor_copy(out=ost, in_=op_ps)
                    for half in range(2):
                        ct = ct0 + cpair * 2 + half
                        nc.sync.dma_start(
                            out=out_d[ct * 128:(ct + 1) * 128,
                                      t4 * 512:(t4 + 1) * 512],
                            in_=ost[:, half * 512:(half + 1) * 512],
                        )

            for qt in range(NQT):
                nkb = 4 * qt + 4
                for hp in range(LH // 2):
                    # interleave the previous qt's out-projection between
                    # head-pairs to keep the PE busy while Act paces the exps
                    if qt > 0:
                        outproj_chunk(qt - 1, 4 * hp)

                    h0 = 2 * hp
                    lkv = h0 // (H // HKV)
                    o_ps = pso.tile([128, 1024], FP32)
                    acc = accpool.tile([128, 1024], BF16)
                    prev = None  # (kb, pt) pending PV/acc
                    for kb in range(nkb):
                        s_ps = pss.tile([128, 1024], FP32, tag="s")
                        for h in range(2):
                            nc.tensor.matmul(
                                s_ps[:, h * 512:(h + 1) * 512],
                                lhsT=kt_all[:, lkv, kb * 128:(kb + 1) * 128],
                                rhs=qt_all[:, h0 + h, qt * 512:(qt + 1) * 512],
                                start=True, stop=True,
                            )
                        pt = ppool.tile([128, 1024], BF16)
                        nc.scalar.activation(pt, s_ps, ACT.Exp, scale=SOFTMAX_SCALE)
                        if kb >= 4 * qt:
                            nc.vector.tensor_mul(pt, pt, mask_s[:, kb - 4 * qt, :])
                        if prev is not None:
                            pkb, ppt = prev
                            for h in range(2):
                                nc.tensor.matmul(
                                    o_ps[:, h * 512:(h + 1) * 512],
                                    lhsT=v_all[:, pkb, lkv * D:(lkv + 1) * D],
                                    rhs=ppt[:, h * 512:(h + 1) * 512],
                                    start=(pkb == 0), stop=False,
                                )
                        if kb == 0:
                            nc.vector.tensor_copy(out=acc, in_=pt)
                        else:
                            nc.vector.tensor_add(acc, acc, pt)
                        prev = (kb, pt)
                    pkb, ppt = prev
                    for h in range(2):
                        nc.tensor.matmul(
                            o_ps[:, h * 512:(h + 1) * 512],
                            lhsT=v_all[:, pkb, lkv * D:(lkv + 1) * D],
                            rhs=ppt[:, h * 512:(h + 1) * 512],
                            start=(pkb == 0), stop=True,
                        )
                    denom = dpool.tile([128, 1024], FP32)
                    nc.gpsimd.partition_all_reduce(
                        denom, acc, channels=128, reduce_op=bass_isa.ReduceOp.add
                    )
                    rcpb = dpool.tile([128, 1024], FP32, tag="rcp")
                    nc.vector.reciprocal(out=rcpb, in_=denom)
                    for h in range(2):
                        nc.vector.tensor_mul(
                            yt_all[:, h0 + h, qt * 512:(qt + 1) * 512],
                            o_ps[:, h * 512:(h + 1) * 512],
                            rcpb[:, h * 512:(h + 1) * 512],
                        )

            # tail: out-projection for the last q-tile
            for ct0 in range(0, NCT, 4):
                outproj_chunk(NQT - 1, ct0)

        persist.release()
        consts.release()

    nc.compile()
    return nc


def _prep_inputs(x, freqs_cis, Wq, Wk, Wv, Wc):
    """Host-side shard + layout prep. Returns the 8 per-core input maps."""
    x = np.asarray(x, dtype=np.float32)
    freqs_cis = np.asarray(freqs_cis, dtype=np.float32)
    NH = LH + LKV

    # cos/sin replicated per local head: [NTT, 128, 2*NH*64]
    cos = np.tile(freqs_cis[:, :, 0], (1, NH))  # [T, NH*64]
    sin = np.tile(freqs_cis[:, :, 1], (1, NH))
    cs = np.concatenate([cos, sin], axis=1).reshape(NTT, 128, 2 * NH * 64)
    cs = np.ascontiguousarray(cs).astype(NPBF16)

    # causal masks for the 4 diagonal alignments, duplicated for head pairs:
    # keep where (y - 128j - x) >= 0
    xg = np.arange(128)[:, None]
    yg = np.arange(512)[None, :]
    masks = np.stack(
        [(yg - 128 * j - xg >= 0) for j in range(4)], axis=1
    ).astype(NPBF16)  # [128, 4, 512]
    masks2 = np.concatenate([masks, masks], axis=2)  # [128, 4, 1024]

    ident = np.eye(128, dtype=NPBF16)

    def tile_rows(w):  # [C, n] -> [128, NCT, n] with row = ct*128 + p
        return np.ascontiguousarray(
            w.reshape(NCT, 128, w.shape[1]).transpose(1, 0, 2)
        ).astype(NPBF16)

    in_maps = []
    xts = []
    for b in range(B):
        xt = x[b].T.reshape(NCT, 128, NTT, 128).transpose(2, 1, 0, 3)
        xts.append(np.ascontiguousarray(xt.reshape(NTT, 128, C)).astype(NPBF16))

    # rotate-half permutation within each head's D columns: new col f gets
    # old col 2f, new col 64+f gets old col 2f+1. Scores are invariant since
    # q and k share the permutation and rope pairs stay matched.
    perm = np.concatenate([np.arange(0, D, 2), np.arange(1, D, 2)])
    Wq = Wq.reshape(C, H, D)[:, :, perm].reshape(C, H * D)
    Wk = Wk.reshape(C, HKV, D)[:, :, perm].reshape(C, HKV * D)

    for core in range(8):
        b, g = core // 2, core % 2
        wq = tile_rows(Wq[:, g * LH * D:(g + 1) * LH * D])
        wk = Wk[:, g * LKV * D:(g + 1) * LKV * D]
        wv = Wv[:, g * LKV * D:(g + 1) * LKV * D]
        wkv = tile_rows(np.concatenate([wk, wv], axis=1))
        wc = np.ascontiguousarray(
            Wc[g * LH * D:(g + 1) * LH * D].reshape(LH, 128, C).transpose(1, 0, 2)
        ).astype(NPBF16)
        in_maps.append(
            {
                "xt": xts[b],
                "wq": wq,
                "wkv": wkv,
                "wc": wc,
                "cs": cs,
                "masks": masks2,
                "ident": ident,
            }
        )
    return in_maps


def kernel(x, freqs_cis, Wq, Wk, Wv, Wc):
    global LAST_RESULTS, _CACHED_NC
    if _CACHED_NC is None:
        _CACHED_NC = _build_module()
    nc = _CACHED_NC
    in_maps = _prep_inputs(x, freqs_cis, Wq, Wk, Wv, Wc)
    res = run_bass_kernel_spmd(nc, in_maps, core_ids=list(range(8)))
    LAST_RESULTS = res
    out = np.empty((B, T, C), dtype=np.float32)
    for b in range(B):
        acc = res.results[2 * b]["outT"] + res.results[2 * b + 1]["outT"]
        out[b] = acc.T
    return out
